# revision 1
# baseline (speedup 1.0000x reference)
"""ClsAttention pooling kernel for 8 TRN2 NeuronCores.

reference:
    att_logits = einsum('bch,nc->bnh', feats, W)      # [B, N, HW]
    att_maps   = softmax(att_logits, axis=2)          # softmax over HW
    cls_feats  = einsum('bnh,bch->bnc', att_maps, feats)

Strategy (data-parallel over batch, 4 items per core, h-halved pipeline):
  - One HBM pass over feats. Each (item, h-half) slab [C, HW/2] is loaded by a
    single SWDGE cast-DMA (f32->fp16) into natural [c, h] layout. Halving the
    pipeline granularity + deep fb buffering keeps the load queue streaming.
  - mm1 weight-stationary: lhsT = W^T chunk [128c, 80], rhs = fb chunk
    [128c, 512h], PSUM accumulate over 4 c-chunks -> logits [80, 512h].
  - exp on ScalarE with accum_out giving softmax partial denominators Z.
  - feats^T and E^T are produced on TensorE (transpose mode) + DVE PSUM->SBUF
    drains. No DMA xbar use at all: xbar_mode would serialize the SDMA engines
    against the streaming cast loads (measured 4us per 640KB E transpose plus
    queue stalls in the xbar variant).
  - mm2: U[80, C] += eT[128h, 80]^T @ ftT[128h, 512c] over 32 h-chunks,
    PSUM bank held across both halves of an item; normalize U/Z, store.
"""

import numpy as np

import concourse.bass as bass
import concourse.mybir as mybir
import concourse.tile as tile
from concourse import bacc
from concourse.bass_utils import run_bass_kernel_spmd
from concourse.masks import make_identity

B, C, HW, NCLS = 32, 512, 4096, 80
NCORES = 8
BPC = B // NCORES   # batch items per core
CCH = C // 128      # c chunks (mm1/mm2 layout)
NS = 2              # h-halves per item (pipeline phases)
HH = HW // NS       # h per half
HB = 512            # h block for mm1 PSUM bank
NHB = HH // HB      # mm1 h-blocks per half
HCH = HH // 128     # h-chunks per half (transpose / mm2 contraction)
FB_BUFS = 5         # fb (cast load) buffers; deep so the DMA streams ahead
FT_BUFS = 3         # transposed-feats buffers
PLP_BUFS = 3        # mm1 logits PSUM banks
PTP_BUFS = 3        # transpose-drain PSUM banks
STORE_ENG = "sync"  # HWDGE ring for output stores
CDT = mybir.dt.float16
F32 = mybir.dt.float32

_cached_nc = None


def _build():
    global _cached_nc
    if _cached_nc is not None:
        return _cached_nc
    nc = bacc.Bacc("TRN2", target_bir_lowering=False, debug=False)
    feats = nc.dram_tensor("feats", [BPC, C, HW], F32, kind="ExternalInput")
    wt = nc.dram_tensor("wt", [C, NCLS], F32, kind="ExternalInput")
    out = nc.dram_tensor("out", [BPC, NCLS, C], F32, kind="ExternalOutput")
    # view for one half-load: [128p, ci, h]
    fview = feats.rearrange("b (ci p) (s h) -> b s p ci h", p=128, s=NS)

    with tile.TileContext(nc) as tc:
        with (
            tc.tile_pool(name="singles", bufs=1) as singles,
            tc.tile_pool(name="fpool", bufs=FB_BUFS) as fpool,
            tc.tile_pool(name="tpool", bufs=FT_BUFS) as tpool,
            tc.tile_pool(name="epool", bufs=2) as epool,
            tc.tile_pool(name="etpool", bufs=2) as etpool,
            tc.tile_pool(name="zpool", bufs=2) as zpool,
            tc.tile_pool(name="opool", bufs=2) as opool,
            tc.tile_pool(name="plp", bufs=PLP_BUFS, space="PSUM") as plp,
            tc.tile_pool(name="pup", bufs=2, space="PSUM") as pup,
            tc.tile_pool(name="ptp", bufs=PTP_BUFS, space="PSUM") as ptp,
        ):
            wt_sb = singles.tile([128, CCH, NCLS], CDT)
            nc.gpsimd.dma_start(
                out=wt_sb, in_=wt.rearrange("(ci p) n -> p ci n", p=128)
            )
            ident = singles.tile([128, 128], CDT)
            make_identity(nc, ident)

            for b in range(BPC):
                zp = zpool.tile([NCLS, NS * NHB], F32)
                pu = pup.tile([NCLS, C], F32)
                for s in range(NS):
                    # load + cast feats[b, half s] to fp16, natural [c, h]
                    fb = fpool.tile([128, CCH, HH], CDT)
                    nc.gpsimd.dma_start(out=fb, in_=fview[b, s])
                    # mm1 (weight stationary) + exp + Z accumulation
                    E = epool.tile([NCLS, HH], CDT)
                    for cb in range(NHB):
                        pl = plp.tile([NCLS, HB], F32)
                        for ci in range(CCH):
                            nc.tensor.matmul(
                                pl,
                                lhsT=wt_sb[:, ci, :],
                                rhs=fb[:, ci, bass.ts(cb, HB)],
                                start=(ci == 0),
                                stop=(ci == CCH - 1),
                            )
                        nc.scalar.activation(
                            out=E[:, bass.ts(cb, HB)],
                            in_=pl,
                            func=mybir.ActivationFunctionType.Exp,
                            accum_out=zp[:, s * NHB + cb : s * NHB + cb + 1],
                        )
                    # feats^T on PE; contiguous slab per c-chunk:
                    # ftT[p, ci, hj, c] = fb[c, hj*128+p]
                    ftT = tpool.tile([128, CCH, HCH, 128], CDT)
                    for ci in range(CCH):
                        for hg in range(HCH // 4):
                            pt = ptp.tile([128, 4, 128], CDT, name="pt", tag="pt")
                            for t in range(4):
                                nc.tensor.transpose(
                                    pt[:, t, :],
                                    fb[:, ci, bass.ts(4 * hg + t, 128)],
                                    ident,
                                )
                            nc.vector.tensor_copy(
                                out=ftT[:, ci, 4 * hg : 4 * hg + 4, :], in_=pt
                            )
                    # E^T on PE: eT[p, hj, n] = E[n, hj*128+p]
                    eT = etpool.tile([128, HCH, NCLS], CDT)
                    for hg in range(HCH // 4):
                        pe_ = ptp.tile([128, 4, NCLS], CDT, name="pe_", tag="pt")
                        for t in range(4):
                            nc.tensor.transpose(
                                pe_[:, t, :],
                                E[:, bass.ts(4 * hg + t, 128)],
                                ident[0:NCLS, 0:NCLS],
                            )
                        nc.vector.tensor_copy(
                            out=eT[:, 4 * hg : 4 * hg + 4, :], in_=pe_
                        )
                    # mm2: U += E^T_chunk^T @ feats^T_chunk over h chunks
                    for hj in range(HCH):
                        nc.tensor.matmul(
                            pu,
                            lhsT=eT[:, hj, :],
                            rhs=ftT[:, :, hj, :],
                            start=(s == 0 and hj == 0),
                            stop=(s == NS - 1 and hj == HCH - 1),
                        )
                # Z, 1/Z, cls = U / Z
                z = zpool.tile([NCLS, 1], F32)
                nc.vector.reduce_sum(z, zp, axis=mybir.AxisListType.X)
                zr = zpool.tile([NCLS, 1], F32)
                nc.vector.reciprocal(zr, z)
                ob = opool.tile([NCLS, C], F32)
                nc.vector.tensor_scalar_mul(ob, pu, zr)
                getattr(nc, STORE_ENG).dma_start(out=out[b], in_=ob)

    nc.compile()
    _cached_nc = nc
    return nc


def kernel(feats: np.ndarray, W: np.ndarray, **run_kwargs) -> np.ndarray:
    nc = _build()
    feats = np.ascontiguousarray(np.asarray(feats), dtype=np.float32)
    wt = np.ascontiguousarray(np.asarray(W, dtype=np.float32).T)
    in_maps = [
        {"feats": np.ascontiguousarray(feats[i * BPC : (i + 1) * BPC]), "wt": wt}
        for i in range(NCORES)
    ]
    res = run_bass_kernel_spmd(nc, in_maps, list(range(NCORES)), **run_kwargs)
    out = np.concatenate([r["out"] for r in res.results], axis=0)
    if run_kwargs:
        kernel.last_results = res
    return np.asarray(out, dtype=np.float32)

